# revision 7
# baseline (speedup 1.0000x reference)
"""Connectome kernel (segment-mean -> Pearson Gram) for 8 TRN2 NeuronCores.

Strategy (pure data parallel, 2 samples per core):
  - Host marshalling: fold mask into parcellation; DROP background /
    masked-out pixels (~50% of V) entirely; sort surviving pixels by ROI
    and pack them into 128-pixel chunks (block B = ROIs 128..199 FIRST,
    then block A = ROIs 0..127; each block padded to a chunk boundary
    with label -1 slots). x is gathered into this packed order, cast
    fp16, laid out [p, chunk, sample, t] per core so each SBUF partition
    reads one contiguous HBM run per chunk-tile. Wire traffic per core:
    ~18.3MB (vs 73.7MB for fp32 all-pixels).
  - Device: stream chunk-tiles on the two HWDGE rings; onehots for ALL
    chunks are built in two batched DVE tensor_tensor ops (is_equal of
    broadcast iota vs broadcast labels); per chunk one PE matmul
    roiT[r, row] += onehot.T @ x_chunk (fp16 operands, fp32 PSUM).
    Block B accumulates first, so its Pearson-normalize chain runs on
    DVE while block A is still streaming.
  - Epilogue per core: the ROI-mean scaling and the +eps in the
    normalizer cancel in the Pearson Gram (normalize(c*s) == normalize(c)
    up to eps ~1e-8 relative), so work directly on the PSUM sums:
    mean + sumsq in two fused passes, 1/norm via reciprocal+sqrt,
    normalized rows emitted fp16, transpose + Gram on PE in fp16,
    write (2,200,200) fp32 conn to HBM.
  - Host: concat cores, extract upper triangle -> (16, 19900).
"""
import sys

sys.path.insert(0, "/opt/trn_rl_repo")

import numpy as np

import concourse.bass as bass
import concourse.tile as tile
from concourse import bacc, mybir
from concourse.bass_utils import run_bass_kernel_spmd

F32 = mybir.dt.float32
F16 = mybir.dt.float16

N, T, H, W = 16, 200, 144, 320
V = H * W                      # 46080
R = 200                        # ROIs
RA = 128                       # ROI block A width (ROIs 0..127)
RB = R - RA                    # ROI block B width (72; ROIs 128..199)
NCORES = 8
SPB = N // NCORES              # samples per core = 2
ROWS = SPB * T                 # 400
CT = 16                        # chunks per DMA tile
EPS = 1e-8                     # cancels in Gram; kept for reference only

_cached = {}


def _bc3(ap2, ins_pos, n):
    """Insert a broadcast (stride 0, count n) dim into a 2D AP."""
    layout = [list(d) for d in ap2.ap]
    layout.insert(ins_pos, [0, n])
    return bass.AP(ap2.tensor, ap2.offset, layout)


def _build_program(nA, nB):
    nch = nA + nB
    nc = bacc.Bacc("TRN2", target_bir_lowering=False, debug=False)

    x_d = nc.declare_dram_parameter("x", [128, nch, ROWS], F16, isOutput=False)
    labs_d = nc.declare_dram_parameter("labs", [128, nch], F16, isOutput=False)
    iota_d = nc.declare_dram_parameter("iota", [128, 128], F16, isOutput=False)
    i128_d = nc.declare_dram_parameter("i128", [128, 128], F16, isOutput=False)
    i72_d = nc.declare_dram_parameter("i72", [72, 72], F16, isOutput=False)
    out_d = nc.declare_dram_parameter("conn", [SPB, R, R], F32, isOutput=True)

    ntiles = (nch + CT - 1) // CT

    with tile.TileContext(nc) as tc:
        with tc.tile_pool(name="consts", bufs=1) as consts, \
             tc.tile_pool(name="loads", bufs=3) as loads, \
             tc.tile_pool(name="ohp", bufs=1) as ohp, \
             tc.tile_pool(name="epi", bufs=1) as epi, \
             tc.tile_pool(name="psum", bufs=1, space="PSUM") as psum:

            labs_s = consts.tile([128, nch], F16)
            iota_s = consts.tile([128, 128], F16)
            i128_s = consts.tile([128, 128], F16)
            i72_s = consts.tile([72, 72], F16)
            # consts go on the gpsimd SW queue: keeps both HWDGE rings
            # (scalar/sync) free for the big x streams from t=0.
            nc.gpsimd.dma_start(labs_s[:], labs_d[:])
            nc.gpsimd.dma_start(iota_s[:], iota_d[:])
            nc.gpsimd.dma_start(i128_s[:], i128_d[:])
            nc.gpsimd.dma_start(i72_s[:], i72_d[:])

            acc_a = psum.tile([RA, ROWS], F32, tag="acc_a", bufs=1)
            acc_b = psum.tile([RB, ROWS], F32, tag="acc_b", bufs=1)

            # All onehots in two batched DVE builds (resident in SBUF).
            ohB = ohp.tile([128, nB, RB], F16, tag="ohB", bufs=1)
            ohA = ohp.tile([128, nA, RA], F16, tag="ohA", bufs=1)
            nc.vector.tensor_tensor(
                ohB[:], _bc3(iota_s[:, 0:RB], 1, nB),
                _bc3(labs_s[:, 0:nB], 2, RB), op=mybir.AluOpType.is_equal)
            nc.vector.tensor_tensor(
                ohA[:], _bc3(iota_s[:, 0:RA], 1, nA),
                _bc3(labs_s[:, nB:nch], 2, RA), op=mybir.AluOpType.is_equal)

            with nc.named_scope("main"):
                for ti in range(ntiles):
                    ch0 = ti * CT
                    ct = min(CT, nch - ch0)
                    ld = loads.tile([128, ct, ROWS], F16,
                                    tag=("ld" if ct == CT else "ld_last"),
                                    bufs=(3 if ct == CT else 1),
                                    name=f"ld_{ti}")
                    eng = nc.scalar if (ti % 2 == 0) else nc.sync
                    eng.dma_start(ld[:], x_d[:, ch0:ch0 + ct, :])

                    for j in range(ct):
                        cc = ch0 + j
                        if cc < nB:
                            acc, oh = acc_b, ohB[:, cc, :]
                            start, stop = (cc == 0), (cc == nB - 1)
                        else:
                            acc, oh = acc_a, ohA[:, cc - nB, :]
                            start, stop = (cc == nB), (cc == nch - 1)
                        nc.tensor.matmul(acc[:], oh, ld[:, j, :],
                                         start=start, stop=stop)

            with nc.named_scope("epilogue"):
                # Pearson-normalize each ROI row directly from the PSUM
                # sums: n = (S - mean) / ||S - mean||.  (1/count scaling
                # and +eps cancel in the normalized Gram.)
                rn = {}
                for blk, acc, P in (("b", acc_b, RB), ("a", acc_a, RA)):
                    Ssb = epi.tile([P, ROWS], F32, tag=f"Ssb_{blk}")
                    nc.vector.tensor_copy(Ssb[:], acc[:])
                    for s in range(SPB):
                        S = Ssb[:, bass.ts(s, T)]
                        tg = f"{blk}{s}"
                        msum = epi.tile([P, 1], F32, tag=f"ms_{tg}")
                        m = epi.tile([P, 1], F32, tag=f"m_{tg}")
                        sq = epi.tile([P, T], F32, tag=f"sq_{blk}",
                                      name=f"sq_{tg}")
                        ssq = epi.tile([P, 1], F32, tag=f"ssq_{tg}")
                        t1 = epi.tile([P, 1], F32, tag=f"t1_{tg}")
                        nrm2 = epi.tile([P, 1], F32, tag=f"n2_{tg}")
                        rr = epi.tile([P, 1], F32, tag=f"rr_{tg}")
                        r_ = epi.tile([P, 1], F32, tag=f"r_{tg}")
                        n16 = epi.tile([P, T], F16, tag=f"n16_{tg}")
                        # ssq = sum S^2 ; msum = sum S  (independent passes)
                        nc.vector.scalar_tensor_tensor(
                            sq[:], S, 1.0, S, op0=mybir.AluOpType.mult,
                            op1=mybir.AluOpType.mult, accum_out=ssq[:])
                        nc.vector.tensor_reduce(msum[:], S,
                                                axis=mybir.AxisListType.X,
                                                op=mybir.AluOpType.add)
                        nc.vector.tensor_scalar_mul(m[:], msum[:], 1.0 / T)
                        # nrm2 = ssq - T*m^2 ; r = 1/sqrt(nrm2)
                        nc.vector.tensor_mul(t1[:], m[:], m[:])
                        nc.vector.scalar_tensor_tensor(
                            nrm2[:], t1[:], -float(T), ssq[:],
                            op0=mybir.AluOpType.mult,
                            op1=mybir.AluOpType.add)
                        nc.vector.reciprocal(rr[:], nrm2[:])
                        nc.scalar.sqrt(r_[:], rr[:])
                        # n = (S - m) * r   (fp16 out)
                        nc.vector.tensor_scalar(n16[:], S, m[:], r_[:],
                                                op0=mybir.AluOpType.subtract,
                                                op1=mybir.AluOpType.mult)
                        rn[(blk, s)] = n16

                for s in range(SPB):
                    # transpose rn -> (t, r) on PE (fp16)
                    trA = psum.tile([128, R], F16, tag="trA", bufs=1,
                                    name=f"trA_{s}")
                    trB = psum.tile([72, R], F16, tag="trB", bufs=1,
                                    name=f"trB_{s}")
                    na, nb = rn[("a", s)], rn[("b", s)]
                    nc.tensor.transpose(trA[:, 0:128], na[:, 0:128], i128_s[:])
                    nc.tensor.transpose(trA[:, 128:200], nb[:, 0:128], i72_s[:])
                    nc.tensor.transpose(trB[:, 0:128], na[:, 128:200], i128_s[:])
                    nc.tensor.transpose(trB[:, 128:200], nb[:, 128:200], i72_s[:])
                    trA_sb = epi.tile([128, R], F16, name=f"trAs_{s}", tag="trAs")
                    trB_sb = epi.tile([72, R], F16, name=f"trBs_{s}", tag="trBs")
                    nc.vector.tensor_copy(trA_sb[:], trA[:])
                    nc.vector.tensor_copy(trB_sb[:], trB[:])

                    # Gram: conn = rn_t.T @ rn_t  (contraction over t, fp16)
                    cA = psum.tile([128, R], F32, tag="cA", bufs=1, name=f"cA_{s}")
                    cB = psum.tile([72, R], F32, tag="cB", bufs=1, name=f"cB_{s}")
                    nc.tensor.matmul(cA[:], trA_sb[:, 0:128], trA_sb[:],
                                     start=True, stop=False)
                    nc.tensor.matmul(cA[:], trB_sb[:, 0:128], trB_sb[:],
                                     start=False, stop=True)
                    nc.tensor.matmul(cB[:], trA_sb[:, 128:200], trA_sb[:],
                                     start=True, stop=False)
                    nc.tensor.matmul(cB[:], trB_sb[:, 128:200], trB_sb[:],
                                     start=False, stop=True)
                    cA_sb = epi.tile([128, R], F32, name=f"cAs_{s}", tag="cAs")
                    cB_sb = epi.tile([72, R], F32, name=f"cBs_{s}", tag="cBs")
                    nc.vector.tensor_copy(cA_sb[:], cA[:])
                    nc.vector.tensor_copy(cB_sb[:], cB[:])
                    nc.sync.dma_start(out_d[s, 0:128, :], cA_sb[:])
                    nc.sync.dma_start(out_d[s, 128:200, :], cB_sb[:])

    nc.compile()
    return nc


def _get_program(nA, nB):
    key = (nA, nB)
    if key not in _cached:
        _cached[key] = _build_program(nA, nB)
    return _cached[key]


def marshal_inputs(x, parc, mask):
    """Host-side prep: packed ROI-sorted fp16 x + tiny derived constants."""
    parc_eff = np.where(np.asarray(mask), np.asarray(parc), 0).reshape(V)
    lab = parc_eff.astype(np.int64) - 1          # -1 = dropped
    counts = np.bincount(parc_eff.astype(np.int64), minlength=R + 1)[1:]

    order = np.argsort(lab, kind="stable")
    nbg = int((lab < 0).sum())
    sorted_idx = order[nbg:]                     # kept pixels, ROI-ascending
    cA = int(counts[0:RA].sum())
    cB = int(counts[RA:R].sum())
    nA = (cA + 127) // 128
    nB = (cB + 127) // 128

    # Block B (ROIs 128..199) first, then block A.
    gB = np.concatenate([sorted_idx[cA:],
                         np.zeros(nB * 128 - cB, dtype=np.int64)])
    gA = np.concatenate([sorted_idx[:cA],
                         np.zeros(nA * 128 - cA, dtype=np.int64)])
    g = np.concatenate([gB, gA])                 # (nch*128,) gather indices
    labB = np.concatenate([lab[sorted_idx[cA:]] - RA,
                           np.full(nB * 128 - cB, -1, dtype=np.int64)])
    labA = np.concatenate([lab[sorted_idx[:cA]],
                           np.full(nA * 128 - cA, -1, dtype=np.int64)])
    nch = nA + nB
    labs = np.concatenate([labB, labA]).astype(np.float16)
    labs = labs.reshape(nch, 128).T.copy()       # (128, nch)

    iota = np.broadcast_to(np.arange(128, dtype=np.float16), (128, 128)).copy()
    i128 = np.eye(128, dtype=np.float16)
    i72 = np.eye(72, dtype=np.float16)

    # (N,1,T,H,W) fp32 -> packed (core, 128, nch, SPB*T) fp16
    x16 = np.asarray(x, dtype=np.float32).reshape(N, T, V).astype(np.float16)
    xg = x16[:, :, g]                            # (N, T, nch*128)
    xg = xg.reshape(NCORES, SPB, T, nch, 128)
    xs = np.ascontiguousarray(xg.transpose(0, 4, 3, 1, 2))  # (8,128,nch,2,T)
    xs = xs.reshape(NCORES, 128, nch, ROWS)

    in_maps = []
    for c in range(NCORES):
        in_maps.append({
            "x": xs[c], "labs": labs, "iota": iota, "i128": i128, "i72": i72,
        })
    return in_maps, nA, nB


def kernel(x, parc, mask):
    in_maps, nA, nB = marshal_inputs(x, parc, mask)
    nc = _get_program(nA, nB)
    res = run_bass_kernel_spmd(nc, in_maps, core_ids=list(range(NCORES)))
    conn = np.concatenate([r["conn"] for r in res.results], axis=0)  # (16,200,200)
    row, col = np.triu_indices(R, k=1)
    return np.ascontiguousarray(conn[:, row, col]).astype(np.float32)


# revision 9
# speedup vs baseline: 1.1477x; 1.1477x over previous
"""Connectome kernel (segment-mean -> Pearson Gram) for 8 TRN2 NeuronCores.

Strategy (pure data parallel, 2 samples per core):
  - Host marshalling: fold mask into parcellation; DROP background /
    masked-out pixels (~50% of V) entirely; sort surviving pixels by ROI
    and pack them into 128-pixel chunks (block B = ROIs 128..199 FIRST,
    then block A = ROIs 0..127; each block padded to a chunk boundary
    with label -1 slots). x is gathered into this packed order, cast
    fp16, laid out [p, chunk, sample, t] per core so each SBUF partition
    reads one contiguous HBM run per chunk-tile. Wire traffic per core:
    ~18.3MB (vs 73.7MB for fp32 all-pixels).
  - Device: stream chunk-tiles on the two HWDGE rings; onehots for ALL
    chunks are built in two batched DVE tensor_tensor ops (is_equal of
    broadcast iota vs broadcast labels); per chunk one PE matmul
    roiT[r, row] += onehot.T @ x_chunk (fp16 operands, fp32 PSUM).
    Block B accumulates first, so its Pearson-normalize chain runs on
    DVE while block A is still streaming.
  - Epilogue per core: the ROI-mean scaling and the +eps in the
    normalizer cancel in the Pearson Gram (normalize(c*s) == normalize(c)
    up to eps ~1e-8 relative), so work directly on the PSUM sums:
    mean + sumsq in two fused passes, 1/norm via reciprocal+sqrt,
    normalized rows emitted fp16, transpose + Gram on PE in fp16,
    write (2,200,200) fp32 conn to HBM.
  - Host: concat cores, extract upper triangle -> (16, 19900).
"""
import sys

sys.path.insert(0, "/opt/trn_rl_repo")

import numpy as np

import concourse.bass as bass
import concourse.tile as tile
from concourse import bacc, mybir
from concourse.bass_utils import run_bass_kernel_spmd

F32 = mybir.dt.float32
F16 = mybir.dt.float16

N, T, H, W = 16, 200, 144, 320
V = H * W                      # 46080
R = 200                        # ROIs
RA = 128                       # ROI block A width (ROIs 0..127)
RB = R - RA                    # ROI block B width (72; ROIs 128..199)
NCORES = 8
SPB = N // NCORES              # samples per core = 2
ROWS = SPB * T                 # 400
CT = 16                        # chunks per DMA tile
EPS = 1e-8                     # cancels in Gram; kept for reference only

_cached = {}


def _bc3(ap2, ins_pos, n):
    """Insert a broadcast (stride 0, count n) dim into a 2D AP."""
    layout = [list(d) for d in ap2.ap]
    layout.insert(ins_pos, [0, n])
    return bass.AP(ap2.tensor, ap2.offset, layout)


def _build_program(nA, nB):
    nch = nA + nB
    nc = bacc.Bacc("TRN2", target_bir_lowering=False, debug=False)

    x_d = nc.declare_dram_parameter("x", [128, nch, ROWS], F16, isOutput=False)
    labs_d = nc.declare_dram_parameter("labs", [128, nch], F16, isOutput=False)
    iota_d = nc.declare_dram_parameter("iota", [128, 128], F16, isOutput=False)
    i128_d = nc.declare_dram_parameter("i128", [128, 128], F16, isOutput=False)
    i72_d = nc.declare_dram_parameter("i72", [72, 72], F16, isOutput=False)
    out_d = nc.declare_dram_parameter("conn", [SPB, R, R], F32, isOutput=True)

    ntiles = (nch + CT - 1) // CT

    with tile.TileContext(nc) as tc:
        with tc.tile_pool(name="consts", bufs=1) as consts, \
             tc.tile_pool(name="loads", bufs=3) as loads, \
             tc.tile_pool(name="ohp", bufs=1) as ohp, \
             tc.tile_pool(name="epi", bufs=1) as epi, \
             tc.tile_pool(name="psum", bufs=1, space="PSUM") as psum:

            labs_s = consts.tile([128, nch], F16)
            iota_s = consts.tile([128, 128], F16)
            i128_s = consts.tile([128, 128], F16)
            i72_s = consts.tile([72, 72], F16)
            # consts first on the sync HWDGE ring (tiny); x tile 0 goes on
            # the scalar ring concurrently, so nothing delays the stream.
            nc.sync.dma_start(labs_s[:], labs_d[:])
            nc.sync.dma_start(iota_s[:], iota_d[:])
            nc.sync.dma_start(i128_s[:], i128_d[:])
            nc.sync.dma_start(i72_s[:], i72_d[:])

            acc_a = psum.tile([RA, ROWS], F32, tag="acc_a", bufs=1)
            acc_b = psum.tile([RB, ROWS], F32, tag="acc_b", bufs=1)

            with nc.named_scope("main"):
                for ti in range(ntiles):
                    ch0 = ti * CT
                    ct = min(CT, nch - ch0)
                    ld = loads.tile([128, ct, ROWS], F16,
                                    tag=("ld" if ct == CT else "ld_last"),
                                    bufs=(8 if ct == CT else 1),
                                    name=f"ld_{ti}")
                    eng = nc.scalar if (ti % 2 == 0) else nc.sync
                    eng.dma_start(ld[:], x_d[:, ch0:ch0 + ct, :])

                    # batched per-tile onehot builds (DVE), one per block
                    # segment present in this tile
                    nb_i = max(0, min(nB, ch0 + ct) - ch0)       # B chunks here
                    na_i = ct - nb_i                             # A chunks here
                    ohB_t = ohA_t = None
                    if nb_i:
                        ohB_t = ohp.tile([128, nb_i, RB], F16,
                                         tag=f"ohB{nb_i}", bufs=3,
                                         name=f"ohB_{ti}")
                        nc.vector.tensor_tensor(
                            ohB_t[:], _bc3(iota_s[:, 0:RB], 1, nb_i),
                            _bc3(labs_s[:, ch0:ch0 + nb_i], 2, RB),
                            op=mybir.AluOpType.is_equal)
                    if na_i:
                        a0 = ch0 + nb_i
                        ohA_t = ohp.tile([128, na_i, RA], F16,
                                         tag=f"ohA{na_i}", bufs=3,
                                         name=f"ohA_{ti}")
                        nc.vector.tensor_tensor(
                            ohA_t[:], _bc3(iota_s[:, 0:RA], 1, na_i),
                            _bc3(labs_s[:, a0:a0 + na_i], 2, RA),
                            op=mybir.AluOpType.is_equal)

                    for j in range(ct):
                        cc = ch0 + j
                        if cc < nB:
                            acc, oh = acc_b, ohB_t[:, j, :]
                            start, stop = (cc == 0), (cc == nB - 1)
                        else:
                            acc, oh = acc_a, ohA_t[:, j - nb_i, :]
                            start, stop = (cc == nB), (cc == nch - 1)
                        nc.tensor.matmul(acc[:], oh, ld[:, j, :],
                                         start=start, stop=stop)

            with nc.named_scope("epilogue"):
                # Pearson-normalize each ROI row directly from the PSUM
                # sums: n = (S - mean) / ||S - mean||.  (1/count scaling
                # and +eps cancel in the normalized Gram.)
                rn = {}
                for blk, acc, P in (("b", acc_b, RB), ("a", acc_a, RA)):
                    Ssb = epi.tile([P, ROWS], F32, tag=f"Ssb_{blk}")
                    nc.vector.tensor_copy(Ssb[:], acc[:])
                    for s in range(SPB):
                        S = Ssb[:, bass.ts(s, T)]
                        tg = f"{blk}{s}"
                        msum = epi.tile([P, 1], F32, tag=f"ms_{tg}")
                        m = epi.tile([P, 1], F32, tag=f"m_{tg}")
                        sq = epi.tile([P, T], F32, tag=f"sq_{blk}",
                                      name=f"sq_{tg}")
                        ssq = epi.tile([P, 1], F32, tag=f"ssq_{tg}")
                        t1 = epi.tile([P, 1], F32, tag=f"t1_{tg}")
                        nrm2 = epi.tile([P, 1], F32, tag=f"n2_{tg}")
                        rr = epi.tile([P, 1], F32, tag=f"rr_{tg}")
                        r_ = epi.tile([P, 1], F32, tag=f"r_{tg}")
                        n16 = epi.tile([P, T], F16, tag=f"n16_{tg}")
                        # ssq = sum S^2 ; msum = sum S  (independent passes)
                        nc.vector.scalar_tensor_tensor(
                            sq[:], S, 1.0, S, op0=mybir.AluOpType.mult,
                            op1=mybir.AluOpType.mult, accum_out=ssq[:])
                        nc.vector.tensor_reduce(msum[:], S,
                                                axis=mybir.AxisListType.X,
                                                op=mybir.AluOpType.add)
                        nc.vector.tensor_scalar_mul(m[:], msum[:], 1.0 / T)
                        # nrm2 = ssq - T*m^2 ; r = 1/sqrt(nrm2)
                        nc.vector.tensor_mul(t1[:], m[:], m[:])
                        nc.vector.scalar_tensor_tensor(
                            nrm2[:], t1[:], -float(T), ssq[:],
                            op0=mybir.AluOpType.mult,
                            op1=mybir.AluOpType.add)
                        nc.vector.reciprocal(rr[:], nrm2[:])
                        nc.scalar.sqrt(r_[:], rr[:])
                        # n = (S - m) * r   (fp16 out)
                        nc.vector.tensor_scalar(n16[:], S, m[:], r_[:],
                                                op0=mybir.AluOpType.subtract,
                                                op1=mybir.AluOpType.mult)
                        rn[(blk, s)] = n16

                for s in range(SPB):
                    # transpose rn -> (t, r) on PE (fp16)
                    trA = psum.tile([128, R], F16, tag="trA", bufs=1,
                                    name=f"trA_{s}")
                    trB = psum.tile([72, R], F16, tag="trB", bufs=1,
                                    name=f"trB_{s}")
                    na, nb = rn[("a", s)], rn[("b", s)]
                    nc.tensor.transpose(trA[:, 0:128], na[:, 0:128], i128_s[:])
                    nc.tensor.transpose(trA[:, 128:200], nb[:, 0:128], i72_s[:])
                    nc.tensor.transpose(trB[:, 0:128], na[:, 128:200], i128_s[:])
                    nc.tensor.transpose(trB[:, 128:200], nb[:, 128:200], i72_s[:])
                    trA_sb = epi.tile([128, R], F16, name=f"trAs_{s}", tag="trAs")
                    trB_sb = epi.tile([72, R], F16, name=f"trBs_{s}", tag="trBs")
                    nc.vector.tensor_copy(trA_sb[:], trA[:])
                    nc.vector.tensor_copy(trB_sb[:], trB[:])

                    # Gram: conn = rn_t.T @ rn_t  (contraction over t, fp16)
                    cA = psum.tile([128, R], F32, tag="cA", bufs=1, name=f"cA_{s}")
                    cB = psum.tile([72, R], F32, tag="cB", bufs=1, name=f"cB_{s}")
                    nc.tensor.matmul(cA[:], trA_sb[:, 0:128], trA_sb[:],
                                     start=True, stop=False)
                    nc.tensor.matmul(cA[:], trB_sb[:, 0:128], trB_sb[:],
                                     start=False, stop=True)
                    nc.tensor.matmul(cB[:], trA_sb[:, 128:200], trA_sb[:],
                                     start=True, stop=False)
                    nc.tensor.matmul(cB[:], trB_sb[:, 128:200], trB_sb[:],
                                     start=False, stop=True)
                    cA_sb = epi.tile([128, R], F32, name=f"cAs_{s}", tag="cAs")
                    cB_sb = epi.tile([72, R], F32, name=f"cBs_{s}", tag="cBs")
                    nc.vector.tensor_copy(cA_sb[:], cA[:])
                    nc.vector.tensor_copy(cB_sb[:], cB[:])
                    nc.sync.dma_start(out_d[s, 0:128, :], cA_sb[:])
                    nc.scalar.dma_start(out_d[s, 128:200, :], cB_sb[:])

    nc.compile()
    return nc


def _get_program(nA, nB):
    key = (nA, nB)
    if key not in _cached:
        _cached[key] = _build_program(nA, nB)
    return _cached[key]


def marshal_inputs(x, parc, mask):
    """Host-side prep: packed ROI-sorted fp16 x + tiny derived constants."""
    parc_eff = np.where(np.asarray(mask), np.asarray(parc), 0).reshape(V)
    lab = parc_eff.astype(np.int64) - 1          # -1 = dropped
    counts = np.bincount(parc_eff.astype(np.int64), minlength=R + 1)[1:]

    order = np.argsort(lab, kind="stable")
    nbg = int((lab < 0).sum())
    sorted_idx = order[nbg:]                     # kept pixels, ROI-ascending
    cA = int(counts[0:RA].sum())
    cB = int(counts[RA:R].sum())
    nA = (cA + 127) // 128
    nB = (cB + 127) // 128

    # Block B (ROIs 128..199) first, then block A.
    gB = np.concatenate([sorted_idx[cA:],
                         np.zeros(nB * 128 - cB, dtype=np.int64)])
    gA = np.concatenate([sorted_idx[:cA],
                         np.zeros(nA * 128 - cA, dtype=np.int64)])
    g = np.concatenate([gB, gA])                 # (nch*128,) gather indices
    labB = np.concatenate([lab[sorted_idx[cA:]] - RA,
                           np.full(nB * 128 - cB, -1, dtype=np.int64)])
    labA = np.concatenate([lab[sorted_idx[:cA]],
                           np.full(nA * 128 - cA, -1, dtype=np.int64)])
    nch = nA + nB
    labs = np.concatenate([labB, labA]).astype(np.float16)
    labs = labs.reshape(nch, 128).T.copy()       # (128, nch)

    iota = np.broadcast_to(np.arange(128, dtype=np.float16), (128, 128)).copy()
    i128 = np.eye(128, dtype=np.float16)
    i72 = np.eye(72, dtype=np.float16)

    # (N,1,T,H,W) fp32 -> packed (core, 128, nch, SPB*T) fp16
    x16 = np.asarray(x, dtype=np.float32).reshape(N, T, V).astype(np.float16)
    xg = x16[:, :, g]                            # (N, T, nch*128)
    xg = xg.reshape(NCORES, SPB, T, nch, 128)
    xs = np.ascontiguousarray(xg.transpose(0, 4, 3, 1, 2))  # (8,128,nch,2,T)
    xs = xs.reshape(NCORES, 128, nch, ROWS)

    in_maps = []
    for c in range(NCORES):
        in_maps.append({
            "x": xs[c], "labs": labs, "iota": iota, "i128": i128, "i72": i72,
        })
    return in_maps, nA, nB


def kernel(x, parc, mask):
    in_maps, nA, nB = marshal_inputs(x, parc, mask)
    nc = _get_program(nA, nB)
    res = run_bass_kernel_spmd(nc, in_maps, core_ids=list(range(NCORES)))
    conn = np.concatenate([r["conn"] for r in res.results], axis=0)  # (16,200,200)
    row, col = np.triu_indices(R, k=1)
    return np.ascontiguousarray(conn[:, row, col]).astype(np.float32)


# revision 15
# speedup vs baseline: 1.2169x; 1.0603x over previous
"""Connectome kernel (segment-mean -> Pearson Gram) for 8 TRN2 NeuronCores.

Strategy (pure data parallel, 2 samples per core):
  - Host marshalling: fold mask into parcellation; DROP background /
    masked-out pixels (~50% of V) entirely; sort surviving pixels by ROI
    and pack them into 128-pixel chunks (block B = ROIs 128..199 FIRST,
    then block A = ROIs 0..127; each block padded to a chunk boundary
    with label -1 slots). x is gathered into this packed order, cast
    fp16, laid out [p, chunk, sample, t] per core so each SBUF partition
    reads one contiguous HBM run per chunk-tile. Wire traffic per core:
    ~18.3MB (vs 73.7MB for fp32 all-pixels).
  - Device: stream chunk-tiles on the two HWDGE rings; onehots for ALL
    chunks are built in two batched DVE tensor_tensor ops (is_equal of
    broadcast iota vs broadcast labels); per chunk one PE matmul
    roiT[r, row] += onehot.T @ x_chunk (fp16 operands, fp32 PSUM).
    Block B accumulates first, so its Pearson-normalize chain runs on
    DVE while block A is still streaming.
  - Epilogue per core: the ROI-mean scaling and the +eps in the
    normalizer cancel in the Pearson Gram (normalize(c*s) == normalize(c)
    up to eps ~1e-8 relative), so work directly on the PSUM sums:
    mean + sumsq in two fused passes, 1/norm via reciprocal+sqrt,
    normalized rows emitted fp16, transpose + Gram on PE in fp16,
    write (2,200,200) fp32 conn to HBM.
  - Host: concat cores, extract upper triangle -> (16, 19900).
"""
import sys

sys.path.insert(0, "/opt/trn_rl_repo")

import numpy as np

import concourse.bass as bass
import concourse.tile as tile
from concourse import bacc, mybir
from concourse.bass_utils import run_bass_kernel_spmd

F32 = mybir.dt.float32
F16 = mybir.dt.float16

N, T, H, W = 16, 200, 144, 320
V = H * W                      # 46080
R = 200                        # ROIs
RA = 128                       # ROI block A width (ROIs 0..127)
RB = R - RA                    # ROI block B width (72; ROIs 128..199)
NCORES = 8
SPB = N // NCORES              # samples per core = 2
ROWS = SPB * T                 # 400
EPS = 1e-8                     # cancels in Gram; kept for reference only


def _tile_sizes(nch):
    """DMA tile schedule: small first tiles to fill the pipe fast, then 8s."""
    sizes, left = [], nch
    while left:
        ct = 4 if len(sizes) < 4 else 8
        ct = min(ct, left)
        sizes.append(ct)
        left -= ct
    return sizes

_cached = {}


def _bc3(ap2, ins_pos, n):
    """Insert a broadcast (stride 0, count n) dim into a 2D AP."""
    layout = [list(d) for d in ap2.ap]
    layout.insert(ins_pos, [0, n])
    return bass.AP(ap2.tensor, ap2.offset, layout)


def _build_program(nA, nB):
    nch = nA + nB
    nc = bacc.Bacc("TRN2", target_bir_lowering=False, debug=False)

    x_d = nc.declare_dram_parameter("x", [128, nch, ROWS], F16, isOutput=False)
    labs_d = nc.declare_dram_parameter("labs", [128, nch], F16, isOutput=False)
    iota_d = nc.declare_dram_parameter("iota", [128, 128], F16, isOutput=False)
    i128_d = nc.declare_dram_parameter("i128", [128, 128], F16, isOutput=False)
    i72_d = nc.declare_dram_parameter("i72", [72, 72], F16, isOutput=False)
    out_d = nc.declare_dram_parameter("conn", [SPB, R, R], F32, isOutput=True)

    tsizes = _tile_sizes(nch)

    with tile.TileContext(nc) as tc:
        with tc.tile_pool(name="consts", bufs=1) as consts, \
             tc.tile_pool(name="loads", bufs=3) as loads, \
             tc.tile_pool(name="ohp", bufs=1) as ohp, \
             tc.tile_pool(name="epi", bufs=1) as epi, \
             tc.tile_pool(name="psum", bufs=1, space="PSUM") as psum:

            labs_s = consts.tile([128, nch], F16)
            iota_s = consts.tile([128, 128], F16)
            i128_s = consts.tile([128, 128], F16)
            i72_s = consts.tile([72, 72], F16)
            # consts first on the sync HWDGE ring (tiny); x tile 0 goes on
            # the scalar ring concurrently, so nothing delays the stream.
            nc.sync.dma_start(labs_s[:], labs_d[:])
            nc.sync.dma_start(iota_s[:], iota_d[:])
            nc.sync.dma_start(i128_s[:], i128_d[:])
            nc.sync.dma_start(i72_s[:], i72_d[:])

            acc_a = psum.tile([RA, ROWS], F32, tag="acc_a", bufs=1)
            acc_b = psum.tile([RB, ROWS], F32, tag="acc_b", bufs=1)

            with nc.named_scope("main"):
                ch0 = 0
                for ti, ct in enumerate(tsizes):
                    ld = loads.tile([128, ct, ROWS], F16, tag=f"ld{ct}",
                                    bufs=(12 if ct == 8 else 4),
                                    name=f"ld_{ti}")
                    eng = nc.scalar if (ti % 2 == 0) else nc.sync
                    eng.dma_start(ld[:], x_d[:, ch0:ch0 + ct, :])

                    # batched per-tile onehot builds (DVE), one per block
                    # segment present in this tile
                    nb_i = max(0, min(nB, ch0 + ct) - ch0)       # B chunks here
                    na_i = ct - nb_i                             # A chunks here
                    ohB_t = ohA_t = None
                    if nb_i:
                        ohB_t = ohp.tile([128, nb_i, RB], F16,
                                         tag=f"ohB{nb_i}", bufs=4,
                                         name=f"ohB_{ti}")
                        nc.vector.tensor_tensor(
                            ohB_t[:], _bc3(iota_s[:, 0:RB], 1, nb_i),
                            _bc3(labs_s[:, ch0:ch0 + nb_i], 2, RB),
                            op=mybir.AluOpType.is_equal)
                    if na_i:
                        a0 = ch0 + nb_i
                        ohA_t = ohp.tile([128, na_i, RA], F16,
                                         tag=f"ohA{na_i}", bufs=4,
                                         name=f"ohA_{ti}")
                        nc.vector.tensor_tensor(
                            ohA_t[:], _bc3(iota_s[:, 0:RA], 1, na_i),
                            _bc3(labs_s[:, a0:a0 + na_i], 2, RA),
                            op=mybir.AluOpType.is_equal)

                    for j in range(ct):
                        cc = ch0 + j
                        if cc < nB:
                            acc, oh = acc_b, ohB_t[:, j, :]
                            start, stop = (cc == 0), (cc == nB - 1)
                        else:
                            acc, oh = acc_a, ohA_t[:, j - nb_i, :]
                            start, stop = (cc == nB), (cc == nch - 1)
                        nc.tensor.matmul(acc[:], oh, ld[:, j, :],
                                         start=start, stop=stop)
                    ch0 += ct

            with nc.named_scope("epilogue"):
                # Pearson-normalize each ROI row directly from the PSUM
                # sums: n = (S - mean) / ||S - mean||.  (1/count scaling
                # and +eps cancel in the normalized Gram.)
                rn = {}
                for blk, acc, P in (("b", acc_b, RB), ("a", acc_a, RA)):
                    Ssb = epi.tile([P, ROWS], F32, tag=f"Ssb_{blk}")
                    nc.vector.tensor_copy(Ssb[:], acc[:])
                    for s in range(SPB):
                        S = Ssb[:, bass.ts(s, T)]
                        tg = f"{blk}{s}"
                        msum = epi.tile([P, 1], F32, tag=f"ms_{tg}")
                        m = epi.tile([P, 1], F32, tag=f"m_{tg}")
                        sq = epi.tile([P, T], F32, tag=f"sq_{blk}",
                                      name=f"sq_{tg}")
                        ssq = epi.tile([P, 1], F32, tag=f"ssq_{tg}")
                        t1 = epi.tile([P, 1], F32, tag=f"t1_{tg}")
                        nrm2 = epi.tile([P, 1], F32, tag=f"n2_{tg}")
                        rr = epi.tile([P, 1], F32, tag=f"rr_{tg}")
                        r_ = epi.tile([P, 1], F32, tag=f"r_{tg}")
                        n16 = epi.tile([P, T], F16, tag=f"n16_{tg}")
                        # ssq = sum S^2 ; msum = sum S  (independent passes)
                        nc.vector.scalar_tensor_tensor(
                            sq[:], S, 1.0, S, op0=mybir.AluOpType.mult,
                            op1=mybir.AluOpType.mult, accum_out=ssq[:])
                        nc.vector.tensor_reduce(msum[:], S,
                                                axis=mybir.AxisListType.X,
                                                op=mybir.AluOpType.add)
                        nc.vector.tensor_scalar_mul(m[:], msum[:], 1.0 / T)
                        # nrm2 = ssq - T*m^2 ; r = 1/sqrt(nrm2)
                        nc.vector.tensor_mul(t1[:], m[:], m[:])
                        nc.vector.scalar_tensor_tensor(
                            nrm2[:], t1[:], -float(T), ssq[:],
                            op0=mybir.AluOpType.mult,
                            op1=mybir.AluOpType.add)
                        nc.vector.reciprocal(rr[:], nrm2[:])
                        nc.scalar.sqrt(r_[:], rr[:])
                        # n = (S - m) * r   (fp16 out)
                        nc.vector.tensor_scalar(n16[:], S, m[:], r_[:],
                                                op0=mybir.AluOpType.subtract,
                                                op1=mybir.AluOpType.mult)
                        rn[(blk, s)] = n16

                for s in range(SPB):
                    # transpose rn -> (t, r) on PE (fp16)
                    trA = psum.tile([128, R], F16, tag="trA", bufs=1,
                                    name=f"trA_{s}")
                    trB = psum.tile([72, R], F16, tag="trB", bufs=1,
                                    name=f"trB_{s}")
                    na, nb = rn[("a", s)], rn[("b", s)]
                    nc.tensor.transpose(trA[:, 0:128], na[:, 0:128], i128_s[:])
                    nc.tensor.transpose(trA[:, 128:200], nb[:, 0:128], i72_s[:])
                    nc.tensor.transpose(trB[:, 0:128], na[:, 128:200], i128_s[:])
                    nc.tensor.transpose(trB[:, 128:200], nb[:, 128:200], i72_s[:])
                    trA_sb = epi.tile([128, R], F16, name=f"trAs_{s}", tag="trAs")
                    trB_sb = epi.tile([72, R], F16, name=f"trBs_{s}", tag="trBs")
                    nc.vector.tensor_copy(trA_sb[:], trA[:])
                    nc.vector.tensor_copy(trB_sb[:], trB[:])

                    # Gram: conn = rn_t.T @ rn_t  (contraction over t, fp16)
                    cA = psum.tile([128, R], F32, tag="cA", bufs=1, name=f"cA_{s}")
                    cB = psum.tile([72, R], F32, tag="cB", bufs=1, name=f"cB_{s}")
                    nc.tensor.matmul(cA[:], trA_sb[:, 0:128], trA_sb[:],
                                     start=True, stop=False)
                    nc.tensor.matmul(cA[:], trB_sb[:, 0:128], trB_sb[:],
                                     start=False, stop=True)
                    nc.tensor.matmul(cB[:], trA_sb[:, 128:200], trA_sb[:],
                                     start=True, stop=False)
                    nc.tensor.matmul(cB[:], trB_sb[:, 128:200], trB_sb[:],
                                     start=False, stop=True)
                    cA_sb = epi.tile([128, R], F32, name=f"cAs_{s}", tag="cAs")
                    cB_sb = epi.tile([72, R], F32, name=f"cBs_{s}", tag="cBs")
                    nc.vector.tensor_copy(cA_sb[:], cA[:])
                    nc.vector.tensor_copy(cB_sb[:], cB[:])
                    nc.sync.dma_start(out_d[s, 0:128, :], cA_sb[:])
                    nc.scalar.dma_start(out_d[s, 128:200, :], cB_sb[:])

    nc.compile()
    return nc


def _get_program(nA, nB):
    key = (nA, nB)
    if key not in _cached:
        _cached[key] = _build_program(nA, nB)
    return _cached[key]


def marshal_inputs(x, parc, mask):
    """Host-side prep: packed ROI-sorted fp16 x + tiny derived constants."""
    parc_eff = np.where(np.asarray(mask), np.asarray(parc), 0).reshape(V)
    lab = parc_eff.astype(np.int64) - 1          # -1 = dropped
    counts = np.bincount(parc_eff.astype(np.int64), minlength=R + 1)[1:]

    order = np.argsort(lab, kind="stable")
    nbg = int((lab < 0).sum())
    sorted_idx = order[nbg:]                     # kept pixels, ROI-ascending
    cA = int(counts[0:RA].sum())
    cB = int(counts[RA:R].sum())
    nA = (cA + 127) // 128
    nB = (cB + 127) // 128

    # Block B (ROIs 128..199) first, then block A.
    gB = np.concatenate([sorted_idx[cA:],
                         np.zeros(nB * 128 - cB, dtype=np.int64)])
    gA = np.concatenate([sorted_idx[:cA],
                         np.zeros(nA * 128 - cA, dtype=np.int64)])
    g = np.concatenate([gB, gA])                 # (nch*128,) gather indices
    labB = np.concatenate([lab[sorted_idx[cA:]] - RA,
                           np.full(nB * 128 - cB, -1, dtype=np.int64)])
    labA = np.concatenate([lab[sorted_idx[:cA]],
                           np.full(nA * 128 - cA, -1, dtype=np.int64)])
    nch = nA + nB
    labs = np.concatenate([labB, labA]).astype(np.float16)
    labs = labs.reshape(nch, 128).T.copy()       # (128, nch)

    iota = np.broadcast_to(np.arange(128, dtype=np.float16), (128, 128)).copy()
    i128 = np.eye(128, dtype=np.float16)
    i72 = np.eye(72, dtype=np.float16)

    # (N,1,T,H,W) fp32 -> packed (core, 128, nch, SPB*T) fp16
    x16 = np.asarray(x, dtype=np.float32).reshape(N, T, V).astype(np.float16)
    xg = x16[:, :, g]                            # (N, T, nch*128)
    xg = xg.reshape(NCORES, SPB, T, nch, 128)
    xs = np.ascontiguousarray(xg.transpose(0, 4, 3, 1, 2))  # (8,128,nch,2,T)
    xs = xs.reshape(NCORES, 128, nch, ROWS)

    in_maps = []
    for c in range(NCORES):
        in_maps.append({
            "x": xs[c], "labs": labs, "iota": iota, "i128": i128, "i72": i72,
        })
    return in_maps, nA, nB


def kernel(x, parc, mask):
    in_maps, nA, nB = marshal_inputs(x, parc, mask)
    nc = _get_program(nA, nB)
    res = run_bass_kernel_spmd(nc, in_maps, core_ids=list(range(NCORES)))
    conn = np.concatenate([r["conn"] for r in res.results], axis=0)  # (16,200,200)
    row, col = np.triu_indices(R, k=1)
    return np.ascontiguousarray(conn[:, row, col]).astype(np.float32)
